# revision 33
# baseline (speedup 1.0000x reference)
"""Trainium2 Bass kernel for LIFNet (leaky-integrator net, no spiking).

Math: the module is linear, and the leaky integration L (a causal LTI filter
along T) commutes with the per-timestep linear layers:

    V2 = L(L(batch @ W1^T) @ W2^T) = (L^2)(batch @ (W2 @ W1)^T)

with Wc = W2 @ W1 of shape [10, 784].  L^2 has impulse response
h[m] = beta^2 (m-1) alpha^(m-2) (m >= 2), which decays below f32 noise by
lag ~128, so the filter is applied as a banded blocked matmul with two
constant 128x128 blocks (intra-block R0, previous-block R1).

The kernel is HBM-bandwidth-bound (the batch read dominates), so:
  - x is pre-cast to fp16 on the host (quantization adds ~4e-4 rel err
    against a 2e-2 gate), halving DMA bytes vs f32.
  - The DRAM layout is per-partition contiguous per (b, t-half), giving
    14 KB descriptors; measured fastest issue pattern under all-8-core
    load is 26 half-b DMAs on the sync HWDGE queue (~250 GB/s/core; the
    f32 4 KB-descriptor version ran at ~207 GB/s).

Device work per core (13 b's, data-parallel over batch; groups of 4 b's
packed 32-partitions apart so downstream stages run 4 b's per instruction):
  - z^T = Wc @ x^T via PE matmuls (fp16, Wc chunks [112, 32] zero-padded,
    tile_position=(0, 32i) places b_i's output rows at psum partition 32i).
  - zp [128, 500] f32 -> zts [128, 2048] fp16 per-(b, tg) cast copies
    (scalar engine) issued right behind each b's matmul burst, so the
    next group's matmuls are not gated on a group-end copy barrier.
  - PE transpose per 128-t-block: [128, 128] -> tpsum fp16; DVE compacts
    the 4x10 used columns into zb slabs [128, 40].
  - V2^T[4 b's] per t'-block via two K=128 fp16 matmuls (R1 prev / R0 cur).
  - v2 [40, 2000] f32 DMA'd out per group on the scalar HWDGE queue.

PE-clock (HAM) management: the tensor engine re-throttles to 1.2 GHz
after any ~3.4us idle window, and transpose-mode matmuls do not count
as activity.  A few dummy matmuls pad each half-b burst and the
transpose bursts so the clock stays at 2.4 GHz.

Tail: the 1-b group is loaded and processed FIRST (its whole pipeline
hides under the other groups' loads); each group's t'-blocks 0-6 are
transposed + filtered as soon as its last b's first half lands; the
last group uses per-b transposes so after the final DMA byte only the
final b's own transposes + the filter remain serial.  Outputs are fp16
(host upcasts) and split per group into [0:896) / [896:2000) slices so
most of the write overlaps the filter.
"""

import sys

import numpy as np

for _p in ("/opt/trn_rl_repo",):
    if _p not in sys.path:
        sys.path.append(_p)

B, T, DIN, H1, H2 = 100, 2000, 784, 100, 10
ALPHA, BETA = 0.7, 0.3

NCORES = 8
BPAD = 104           # batch padded to 8 * 13
BP = BPAD // NCORES  # 13 b's per core
DC = 112             # d-chunk width (784 = 7 * 112), partition dim of x tiles
NDC = DIN // DC      # 7
MP = 32              # padded output rows per b (10 real + 22 zero)
TH = T // 2          # t-half
TG = 500             # t-columns per z-matmul group (one psum bank)
NTG = T // TG        # 4
TB = 128             # t'-block for the filter stage
NTB = (T + TB - 1) // TB   # 16
TPADF = NTB * TB     # 2048 free-dim padding for the z^T staging buffer
# Singleton first: its data arrives in the first two DMA slots and its
# whole pipeline overlaps the other groups' loads, so the kernel tail is
# only the last 4-group's second filter phase.
GROUPS = [(12, 1), (0, 4), (4, 4), (8, 4)]  # (first b, group size)
NWARM = 4            # dummy N=500 matmuls appended per half-b burst

_CACHE: dict = {}


def _filter_blocks() -> np.ndarray:
    """R = [R1 | R0] as [128, 256] fp16: rhs blocks for the filter matmuls.

    out[o, t'] += sum_tl z_block[tl, o] * R[tl, t'] with R[tl, t'] =
    h[lag], lag = (t' - tl) + 128 for R1 (z from previous t-block) and
    (t' - tl) for R0 (intra-block, strictly causal).
    """
    m = np.arange(512, dtype=np.float64)
    h = np.zeros(512)
    h[2:] = BETA * BETA * (m[2:] - 1.0) * ALPHA ** (m[2:] - 2.0)
    tl = np.arange(TB)[:, None]
    tp = np.arange(TB)[None, :]
    r1 = h[tp - tl + TB]
    lag0 = tp - tl
    r0 = np.where(lag0 >= 2, h[np.clip(lag0, 0, None)], 0.0)
    return np.concatenate([r1, r0], axis=1).astype(np.float16)


def _build(reps: int = 1):
    """Build + compile the per-core Bass kernel (shared by all 8 cores)."""
    from contextlib import ExitStack

    import concourse.tile as tile
    from concourse import bacc, mybir

    f16 = mybir.dt.float16
    f32 = mybir.dt.float32
    nc = bacc.Bacc(
        "TRN2", target_bir_lowering=False, debug=False, num_devices=NCORES
    )

    # per-partition layout per b: [half h][chunk c][t' in half] (7000 each)
    xT = nc.dram_tensor("xT", [BP, DC, NDC * T], f16, kind="ExternalInput")
    wct = nc.dram_tensor("wct", [DC, NDC * MP], f16, kind="ExternalInput")
    rh = nc.dram_tensor("rh", [TB, 2 * TB], f16, kind="ExternalInput")
    eye = nc.dram_tensor("eye", [TB, TB], f16, kind="ExternalInput")
    vout = nc.dram_tensor("vout", [BP * H2, T], f16, kind="ExternalOutput")

    with tile.TileContext(nc) as tc, ExitStack() as ctx:
        const = ctx.enter_context(tc.tile_pool(name="const", bufs=1))
        xpool = ctx.enter_context(tc.tile_pool(name="xp", bufs=10))
        ring = ctx.enter_context(tc.tile_pool(name="ring", bufs=1))
        zbp = ctx.enter_context(tc.tile_pool(name="zbp", bufs=2))
        vsb = ctx.enter_context(tc.tile_pool(name="vsb", bufs=2))
        zpsum = ctx.enter_context(tc.tile_pool(name="zps", bufs=1, space="PSUM"))
        tpsum = ctx.enter_context(tc.tile_pool(name="tps", bufs=2, space="PSUM"))
        vpsum = ctx.enter_context(tc.tile_pool(name="vps", bufs=1, space="PSUM"))
        dpsum = ctx.enter_context(tc.tile_pool(name="dps", bufs=1, space="PSUM"))

        # consts on the scalar HWDGE queue so they don't delay the first
        # x load on the sync queue
        wct_sb = const.tile([DC, NDC * MP], f16, tag="wct")
        nc.scalar.dma_start(wct_sb[:], wct.ap())
        rh_sb = const.tile([TB, 2 * TB], f16, tag="rh")
        nc.scalar.dma_start(rh_sb[:], rh.ap())
        eye_sb = const.tile([TB, TB], f16, tag="eye")
        nc.scalar.dma_start(eye_sb[:], eye.ap())

        # Two-deep manual ring: the t-pad cols (>=2000) of the z^T staging
        # tile must stay zero across groups, so memset only once.
        zts_ring = []
        for i in range(2):
            zt = ring.tile([TB, TPADF], f16, tag=f"zts{i}", name=f"zts{i}")
            nc.vector.memset(zt[:], 0.0)
            zts_ring.append(zt)

        def warm(xv, n=NWARM):
            """Dummy matmuls: count as PE activity for the HAM clock gate."""
            dmy = dpsum.tile([1, TG], f32, tag="dmy", name="dmy")
            for _ in range(n):
                nc.tensor.matmul(
                    dmy[:], wct_sb[:, 0:1], xv[:, 0, 0:TG],
                    start=True, stop=True,
                )

        def z_half(zp_tiles, xv, i, h, copy_rows, zts, nwarm=NWARM):
            """One half-b of stage-1 matmuls + its two zts cast copies."""
            for tg in (0, 1):
                zp = zp_tiles[2 * h + tg]
                for c in range(NDC):
                    nc.tensor.matmul(
                        zp[MP * i : MP * (i + 1), :],
                        wct_sb[:, c * MP : (c + 1) * MP],
                        xv[:, c, tg * TG : (tg + 1) * TG],
                        start=(c == 0),
                        stop=(c == NDC - 1),
                        tile_position=(0, MP * i),
                    )
            warm(xv, nwarm)
            r0, r1 = copy_rows
            for tg in (0, 1):
                gtg = 2 * h + tg
                nc.scalar.copy(
                    zts[r0:r1, gtg * TG : (gtg + 1) * TG],
                    zp_tiles[gtg][r0:r1, :],
                )

        def transposes(zts, zbv, G, xv_warm, j0, j1, per_b=None):
            """z^T -> zb for t'-blocks [j0, j1).

            per_b=i transposes only b_i's 32-partition band (so the last
            group's final b leaves just its own transposes for the tail).
            """
            for j in range(j0, j1):
                tp = tpsum.tile([TB, TB], f16, tag="tp", name="tp")
                if per_b is None:
                    nc.tensor.transpose(
                        tp[:], zts[:, j * TB : (j + 1) * TB], eye_sb[:]
                    )
                    tpv = tp[:].rearrange("p (gg o) -> p gg o", gg=4)
                    nc.vector.tensor_copy(
                        zbv[:, j, 0:G, :], tpv[:, 0:G, 0:H2]
                    )
                else:
                    i = per_b
                    nc.tensor.transpose(
                        tp[:, 0:MP],
                        zts[MP * i : MP * (i + 1), j * TB : (j + 1) * TB],
                        eye_sb[MP * i : MP * (i + 1), MP * i : MP * (i + 1)],
                        tile_position=(MP * i, 0),
                    )
                    nc.vector.tensor_copy(
                        zbv[:, j, i, :], tp[:, 0:H2]
                    )
                if j % 4 == 3 and xv_warm is not None:
                    # transpose-mode matmuls don't register as PE activity
                    # for the clock gate; sprinkle a real one
                    dmy = dpsum.tile([1, TG], f32, tag="dmy", name="dmy")
                    nc.tensor.matmul(
                        dmy[:], wct_sb[:, 0:1], xv_warm[:, 0, 0:TG],
                        start=True, stop=True,
                    )

        def stage23(zts, zb, zbv, v2, G, OG, xv_warm, j0, j1, skip_t=False):
            """Transpose + filter for t'-blocks [j0, j1)."""
            if not skip_t:
                transposes(zts, zbv, G, xv_warm, j0, j1)
            for j in range(j0, j1):
                vp = vpsum.tile([4 * H2, TB], f32, tag="vp", name="vp")
                n_mm = 2 if j > 0 else 1
                mm = 0
                for roff, jj in ((0, j - 1), (TB, j)):
                    if jj < 0:
                        continue
                    nc.tensor.matmul(
                        vp[0:OG, :],
                        zb[:, jj * 4 * H2 : jj * 4 * H2 + OG],
                        rh_sb[:, roff : roff + TB],
                        start=(mm == 0),
                        stop=(mm == n_mm - 1),
                    )
                    mm += 1
                w = min(TB, T - j * TB)
                nc.vector.tensor_copy(
                    v2[0:OG, j * TB : j * TB + w], vp[0:OG, 0:w]
                )

        for rep in range(reps):
          for g, (b0, G) in enumerate(GROUPS):
            zts = zts_ring[g % 2]
            last_grp = g == len(GROUPS) - 1

            zp_tiles = [
                zpsum.tile([TB, TG], f32, tag=f"zp{tg}", name=f"zp{tg}")
                for tg in range(NTG)
            ]
            zb = zbp.tile([TB, NTB * 4 * H2], f16, tag="zb")
            zbv = zb[:].rearrange("p (j gg o) -> p j gg o", j=NTB, gg=4)
            v2 = vsb.tile([4 * H2, T], f16, tag="v2")
            OG = H2 * G

            for i in range(G):
                b = b0 + i
                rows = (MP * i, MP * (i + 1))
                for h in range(2):
                    xt = xpool.tile([DC, NDC * TH], f16, tag="xt")
                    xv = xt[:].rearrange("p (c t) -> p c t", c=NDC)
                    nc.sync.dma_start(
                        xt[:],
                        xT.ap()[b, :, h * NDC * TH : (h + 1) * NDC * TH],
                    )
                    z_half(zp_tiles, xv, i, h, rows, zts)
                    if i == G - 1 and h == 0:
                        # t'-blocks 0-6 only need t < 896: transpose +
                        # filter them while the last half-b streams in
                        stage23(zts, zb, zbv, v2, G, OG, xv, 0, 7)
                        nc.scalar.dma_start(
                            vout.ap()[H2 * b0 : H2 * b0 + OG, 0 : 7 * TB],
                            v2[0:OG, 0 : 7 * TB],
                        )
                    if last_grp and h == 1:
                        # per-b transposes as each b's second half finishes:
                        # after the final DMA only the last b's own
                        # transposes and filter remain on the critical path
                        transposes(zts, zbv, G, xv, 7, NTB, per_b=i)
            stage23(zts, zb, zbv, v2, G, OG, xv, 7, NTB, skip_t=last_grp)
            # final slice on the sync queue: it is idle by now, while the
            # scalar engine is still draining copies ahead of its queue
            outq = nc.sync if g == len(GROUPS) - 1 else nc.scalar
            outq.dma_start(
                vout.ap()[H2 * b0 : H2 * b0 + OG, 7 * TB : T],
                v2[0:OG, 7 * TB : T],
            )

    nc.compile()
    return nc


def _prep_inputs(batch: np.ndarray, W1: np.ndarray, W2: np.ndarray):
    wc = (W2.astype(np.float64) @ W1.astype(np.float64)).astype(np.float32)
    # [112, 7, 32]: wct[p, c, o] = Wc[o, 112c + p] for o < 10, else 0
    wct = np.zeros((DC, NDC, MP), np.float16)
    wct[:, :, :H2] = wc.T.reshape(NDC, DC, H2).transpose(1, 0, 2)
    wct = np.ascontiguousarray(wct.reshape(DC, NDC * MP))
    rh = _filter_blocks()
    eye = np.eye(TB, dtype=np.float16)

    bp = np.zeros((BPAD, T, DIN), np.float16)
    bp[:B] = batch.astype(np.float16)
    # [8, 13, 112, 2, 7, 1000]: per-(b, half) partition runs of 14 KB
    xt = np.ascontiguousarray(
        bp.reshape(NCORES, BP, 2, TH, NDC, DC).transpose(0, 1, 5, 2, 4, 3)
    ).reshape(NCORES, BP, DC, NDC * T)
    return xt, wct, rh, eye


def kernel(batch: np.ndarray, W1: np.ndarray, W2: np.ndarray) -> np.ndarray:
    from concourse import bass_utils

    if "nc" not in _CACHE:
        _CACHE["nc"] = _build()
    nc = _CACHE["nc"]

    xt, wct, rh, eye = _prep_inputs(batch, W1, W2)
    in_maps = [
        {"xT": xt[i], "wct": wct, "rh": rh, "eye": eye} for i in range(NCORES)
    ]
    res = bass_utils.run_bass_kernel_spmd(
        nc, in_maps, core_ids=list(range(NCORES)), **_CACHE.get("run_kwargs", {})
    )
    _CACHE["last_result"] = res

    full = np.concatenate(
        [r["vout"].reshape(BP, H2, T) for r in res.results], axis=0
    )  # [104, 10, 2000]
    return np.ascontiguousarray(full.transpose(0, 2, 1)[:B].astype(np.float32))


# revision 34
# speedup vs baseline: 1.0237x; 1.0237x over previous
"""Trainium2 Bass kernel for LIFNet (leaky-integrator net, no spiking).

Math: the module is linear, and the leaky integration L (a causal LTI filter
along T) commutes with the per-timestep linear layers:

    V2 = L(L(batch @ W1^T) @ W2^T) = (L^2)(batch @ (W2 @ W1)^T)

with Wc = W2 @ W1 of shape [10, 784].  L^2 has impulse response
h[m] = beta^2 (m-1) alpha^(m-2) (m >= 2), which decays below f32 noise by
lag ~128, so the filter is applied as a banded blocked matmul with two
constant 128x128 blocks (intra-block R0, previous-block R1).

The kernel is HBM-bandwidth-bound (the batch read dominates), so:
  - x is pre-cast to fp16 on the host (quantization adds ~4e-4 rel err
    against a 2e-2 gate), halving DMA bytes vs f32.
  - The DRAM layout is per-partition contiguous per (b, t-half), giving
    14 KB descriptors; measured fastest issue pattern under all-8-core
    load is 26 half-b DMAs on the sync HWDGE queue (~250 GB/s/core; the
    f32 4 KB-descriptor version ran at ~207 GB/s).

Device work per core (13 b's, data-parallel over batch; groups of 4 b's
packed 32-partitions apart so downstream stages run 4 b's per instruction):
  - z^T = Wc @ x^T via PE matmuls (fp16, Wc chunks [112, 32] zero-padded,
    tile_position=(0, 32i) places b_i's output rows at psum partition 32i).
  - zp [128, 500] f32 -> zts [128, 2048] fp16 per-(b, tg) cast copies
    (scalar engine) issued right behind each b's matmul burst, so the
    next group's matmuls are not gated on a group-end copy barrier.
  - PE transpose per 128-t-block: [128, 128] -> tpsum fp16; DVE compacts
    the 4x10 used columns into zb slabs [128, 40].
  - V2^T[4 b's] per t'-block via two K=128 fp16 matmuls (R1 prev / R0 cur).
  - v2 [40, 2000] f32 DMA'd out per group on the scalar HWDGE queue.

PE-clock (HAM) management: the tensor engine re-throttles to 1.2 GHz
after any ~3.4us idle window, and transpose-mode matmuls do not count
as activity.  A few dummy matmuls pad each half-b burst and the
transpose bursts so the clock stays at 2.4 GHz.

Tail: the 1-b group is loaded and processed FIRST (its whole pipeline
hides under the other groups' loads); each group's t'-blocks 0-6 are
transposed + filtered as soon as its last b's first half lands; the
last group uses per-b transposes so after the final DMA byte only the
final b's own transposes + the filter remain serial.  Outputs are fp16
(host upcasts) and split per group into [0:896) / [896:2000) slices so
most of the write overlaps the filter.
"""

import sys

import numpy as np

for _p in ("/opt/trn_rl_repo",):
    if _p not in sys.path:
        sys.path.append(_p)

B, T, DIN, H1, H2 = 100, 2000, 784, 100, 10
ALPHA, BETA = 0.7, 0.3

NCORES = 8
BPAD = 104           # batch padded to 8 * 13
BP = BPAD // NCORES  # 13 b's per core
DC = 112             # d-chunk width (784 = 7 * 112), partition dim of x tiles
NDC = DIN // DC      # 7
MP = 32              # padded output rows per b (10 real + 22 zero)
TH = T // 2          # t-half
TG = 500             # t-columns per z-matmul group (one psum bank)
NTG = T // TG        # 4
TB = 128             # t'-block for the filter stage
NTB = (T + TB - 1) // TB   # 16
TPADF = NTB * TB     # 2048 free-dim padding for the z^T staging buffer
# Singleton first: its data arrives in the first two DMA slots and its
# whole pipeline overlaps the other groups' loads, so the kernel tail is
# only the last 4-group's second filter phase.
GROUPS = [(12, 1), (0, 4), (4, 4), (8, 4)]  # (first b, group size)
NWARM = 4            # dummy N=500 matmuls appended per half-b burst

_CACHE: dict = {}


def _filter_blocks() -> np.ndarray:
    """R = [R1 | R0] as [128, 256] fp16: rhs blocks for the filter matmuls.

    out[o, t'] += sum_tl z_block[tl, o] * R[tl, t'] with R[tl, t'] =
    h[lag], lag = (t' - tl) + 128 for R1 (z from previous t-block) and
    (t' - tl) for R0 (intra-block, strictly causal).
    """
    m = np.arange(512, dtype=np.float64)
    h = np.zeros(512)
    h[2:] = BETA * BETA * (m[2:] - 1.0) * ALPHA ** (m[2:] - 2.0)
    tl = np.arange(TB)[:, None]
    tp = np.arange(TB)[None, :]
    r1 = h[tp - tl + TB]
    lag0 = tp - tl
    r0 = np.where(lag0 >= 2, h[np.clip(lag0, 0, None)], 0.0)
    return np.concatenate([r1, r0], axis=1).astype(np.float16)


def _build(reps: int = 1):
    """Build + compile the per-core Bass kernel (shared by all 8 cores)."""
    from contextlib import ExitStack

    import concourse.tile as tile
    from concourse import bacc, mybir

    f16 = mybir.dt.float16
    f32 = mybir.dt.float32
    nc = bacc.Bacc(
        "TRN2", target_bir_lowering=False, debug=False, num_devices=NCORES
    )

    # per-partition layout per b: [half h][chunk c][t' in half] (7000 each)
    xT = nc.dram_tensor("xT", [BP, DC, NDC * T], f16, kind="ExternalInput")
    wct = nc.dram_tensor("wct", [DC, NDC * MP], f16, kind="ExternalInput")
    rh = nc.dram_tensor("rh", [TB, 2 * TB], f16, kind="ExternalInput")
    eye = nc.dram_tensor("eye", [TB, TB], f16, kind="ExternalInput")
    vout = nc.dram_tensor("vout", [BP * H2, T], f16, kind="ExternalOutput")

    with tile.TileContext(nc) as tc, ExitStack() as ctx:
        const = ctx.enter_context(tc.tile_pool(name="const", bufs=1))
        xpool = ctx.enter_context(tc.tile_pool(name="xp", bufs=10))
        ring = ctx.enter_context(tc.tile_pool(name="ring", bufs=1))
        zbp = ctx.enter_context(tc.tile_pool(name="zbp", bufs=2))
        vsb = ctx.enter_context(tc.tile_pool(name="vsb", bufs=2))
        zpsum = ctx.enter_context(tc.tile_pool(name="zps", bufs=1, space="PSUM"))
        tpsum = ctx.enter_context(tc.tile_pool(name="tps", bufs=2, space="PSUM"))
        vpsum = ctx.enter_context(tc.tile_pool(name="vps", bufs=1, space="PSUM"))
        dpsum = ctx.enter_context(tc.tile_pool(name="dps", bufs=1, space="PSUM"))

        # consts on the scalar HWDGE queue so they don't delay the first
        # x load on the sync queue
        wct_sb = const.tile([DC, NDC * MP], f16, tag="wct")
        nc.scalar.dma_start(wct_sb[:], wct.ap())
        rh_sb = const.tile([TB, 2 * TB], f16, tag="rh")
        nc.scalar.dma_start(rh_sb[:], rh.ap())
        eye_sb = const.tile([TB, TB], f16, tag="eye")
        nc.scalar.dma_start(eye_sb[:], eye.ap())

        # Two-deep manual ring: the t-pad cols (>=2000) of the z^T staging
        # tile must stay zero across groups, so memset only once.
        zts_ring = []
        for i in range(2):
            zt = ring.tile([TB, TPADF], f16, tag=f"zts{i}", name=f"zts{i}")
            nc.vector.memset(zt[:], 0.0)
            zts_ring.append(zt)

        def warm(xv, n=NWARM):
            """Dummy matmuls: count as PE activity for the HAM clock gate."""
            dmy = dpsum.tile([1, TG], f32, tag="dmy", name="dmy")
            for _ in range(n):
                nc.tensor.matmul(
                    dmy[:], wct_sb[:, 0:1], xv[:, 0, 0:TG],
                    start=True, stop=True,
                )

        def z_half(zp_tiles, xv, i, h, copy_rows, zts, nwarm=NWARM):
            """One half-b of stage-1 matmuls + its two zts cast copies."""
            for tg in (0, 1):
                zp = zp_tiles[2 * h + tg]
                for c in range(NDC):
                    nc.tensor.matmul(
                        zp[MP * i : MP * (i + 1), :],
                        wct_sb[:, c * MP : (c + 1) * MP],
                        xv[:, c, tg * TG : (tg + 1) * TG],
                        start=(c == 0),
                        stop=(c == NDC - 1),
                        tile_position=(0, MP * i),
                    )
            warm(xv, nwarm)
            r0, r1 = copy_rows
            for tg in (0, 1):
                gtg = 2 * h + tg
                nc.scalar.copy(
                    zts[r0:r1, gtg * TG : (gtg + 1) * TG],
                    zp_tiles[gtg][r0:r1, :],
                )

        def transposes(zts, zbv, G, xv_warm, j0, j1, per_b=None):
            """z^T -> zb for t'-blocks [j0, j1).

            per_b=i transposes only b_i's 32-partition band (so the last
            group's final b leaves just its own transposes for the tail).
            """
            for j in range(j0, j1):
                tp = tpsum.tile([TB, TB], f16, tag="tp", name="tp")
                if per_b is None:
                    nc.tensor.transpose(
                        tp[:], zts[:, j * TB : (j + 1) * TB], eye_sb[:]
                    )
                    tpv = tp[:].rearrange("p (gg o) -> p gg o", gg=4)
                    nc.vector.tensor_copy(
                        zbv[:, j, 0:G, :], tpv[:, 0:G, 0:H2]
                    )
                else:
                    i = per_b
                    nc.tensor.transpose(
                        tp[:, 0:MP],
                        zts[MP * i : MP * (i + 1), j * TB : (j + 1) * TB],
                        eye_sb[MP * i : MP * (i + 1), MP * i : MP * (i + 1)],
                        tile_position=(MP * i, 0),
                    )
                    nc.vector.tensor_copy(
                        zbv[:, j, i, :], tp[:, 0:H2]
                    )
                if j % 4 == 3 and xv_warm is not None:
                    # transpose-mode matmuls don't register as PE activity
                    # for the clock gate; sprinkle a real one
                    dmy = dpsum.tile([1, TG], f32, tag="dmy", name="dmy")
                    nc.tensor.matmul(
                        dmy[:], wct_sb[:, 0:1], xv_warm[:, 0, 0:TG],
                        start=True, stop=True,
                    )

        def stage23(zts, zb, zbv, v2, G, OG, xv_warm, j0, j1, skip_t=False):
            """Transpose + filter for t'-blocks [j0, j1)."""
            if not skip_t:
                transposes(zts, zbv, G, xv_warm, j0, j1)
            for j in range(j0, j1):
                vp = vpsum.tile([4 * H2, TB], f32, tag="vp", name="vp")
                n_mm = 2 if j > 0 else 1
                mm = 0
                for roff, jj in ((0, j - 1), (TB, j)):
                    if jj < 0:
                        continue
                    nc.tensor.matmul(
                        vp[0:OG, :],
                        zb[:, jj * 4 * H2 : jj * 4 * H2 + OG],
                        rh_sb[:, roff : roff + TB],
                        start=(mm == 0),
                        stop=(mm == n_mm - 1),
                    )
                    mm += 1
                w = min(TB, T - j * TB)
                nc.vector.tensor_copy(
                    v2[0:OG, j * TB : j * TB + w], vp[0:OG, 0:w]
                )

        for rep in range(reps):
          for g, (b0, G) in enumerate(GROUPS):
            zts = zts_ring[g % 2]
            last_grp = g == len(GROUPS) - 1

            zp_tiles = [
                zpsum.tile([TB, TG], f32, tag=f"zp{tg}", name=f"zp{tg}")
                for tg in range(NTG)
            ]
            zb = zbp.tile([TB, NTB * 4 * H2], f16, tag="zb")
            zbv = zb[:].rearrange("p (j gg o) -> p j gg o", j=NTB, gg=4)
            v2 = vsb.tile([4 * H2, T], f16, tag="v2")
            OG = H2 * G

            for i in range(G):
                b = b0 + i
                rows = (MP * i, MP * (i + 1))
                for h in range(2):
                    xt = xpool.tile([DC, NDC * TH], f16, tag="xt")
                    xv = xt[:].rearrange("p (c t) -> p c t", c=NDC)
                    if last_grp and i == G - 1 and h == 1:
                        # the very last load, split into d-chunk pieces with
                        # c-outer matmuls: the serial tail only waits on the
                        # final [112, 2x1000] piece (~0.45 MB), not 1.57 MB
                        bounds = [0, 2, 4, 6, NDC]
                        for k in range(4):
                            c0, c1 = bounds[k], bounds[k + 1]
                            nc.sync.dma_start(
                                xt[:, c0 * TH : c1 * TH],
                                xT.ap()[
                                    b, :,
                                    NDC * TH + c0 * TH : NDC * TH + c1 * TH,
                                ],
                            )
                        for c in range(NDC):
                            for tg in (0, 1):
                                nc.tensor.matmul(
                                    zp_tiles[2 + tg][rows[0] : rows[1], :],
                                    wct_sb[:, c * MP : (c + 1) * MP],
                                    xv[:, c, tg * TG : (tg + 1) * TG],
                                    start=(c == 0),
                                    stop=(c == NDC - 1),
                                    tile_position=(0, MP * i),
                                )
                        for tg in (2, 3):
                            nc.scalar.copy(
                                zts[rows[0] : rows[1], tg * TG : (tg + 1) * TG],
                                zp_tiles[tg][rows[0] : rows[1], :],
                            )
                        transposes(zts, zbv, G, xv, 7, NTB, per_b=i)
                        continue
                    nc.sync.dma_start(
                        xt[:],
                        xT.ap()[b, :, h * NDC * TH : (h + 1) * NDC * TH],
                    )
                    z_half(zp_tiles, xv, i, h, rows, zts,
                           nwarm=8 if (last_grp and i == G - 1) else NWARM)
                    if i == G - 1 and h == 0:
                        # t'-blocks 0-6 only need t < 896: transpose +
                        # filter them while the last half-b streams in
                        stage23(zts, zb, zbv, v2, G, OG, xv, 0, 7)
                        nc.scalar.dma_start(
                            vout.ap()[H2 * b0 : H2 * b0 + OG, 0 : 7 * TB],
                            v2[0:OG, 0 : 7 * TB],
                        )
                    if last_grp and h == 1:
                        # per-b transposes as each b's second half finishes:
                        # after the final DMA only the last b's own
                        # transposes and filter remain on the critical path
                        transposes(zts, zbv, G, xv, 7, NTB, per_b=i)
            stage23(zts, zb, zbv, v2, G, OG, xv, 7, NTB, skip_t=last_grp)
            # final slice on the sync queue: it is idle by now, while the
            # scalar engine is still draining copies ahead of its queue
            outq = nc.sync if g == len(GROUPS) - 1 else nc.scalar
            outq.dma_start(
                vout.ap()[H2 * b0 : H2 * b0 + OG, 7 * TB : T],
                v2[0:OG, 7 * TB : T],
            )

    nc.compile()
    return nc


def _prep_inputs(batch: np.ndarray, W1: np.ndarray, W2: np.ndarray):
    wc = (W2.astype(np.float64) @ W1.astype(np.float64)).astype(np.float32)
    # [112, 7, 32]: wct[p, c, o] = Wc[o, 112c + p] for o < 10, else 0
    wct = np.zeros((DC, NDC, MP), np.float16)
    wct[:, :, :H2] = wc.T.reshape(NDC, DC, H2).transpose(1, 0, 2)
    wct = np.ascontiguousarray(wct.reshape(DC, NDC * MP))
    rh = _filter_blocks()
    eye = np.eye(TB, dtype=np.float16)

    bp = np.zeros((BPAD, T, DIN), np.float16)
    bp[:B] = batch.astype(np.float16)
    # [8, 13, 112, 2, 7, 1000]: per-(b, half) partition runs of 14 KB
    xt = np.ascontiguousarray(
        bp.reshape(NCORES, BP, 2, TH, NDC, DC).transpose(0, 1, 5, 2, 4, 3)
    ).reshape(NCORES, BP, DC, NDC * T)
    return xt, wct, rh, eye


def kernel(batch: np.ndarray, W1: np.ndarray, W2: np.ndarray) -> np.ndarray:
    from concourse import bass_utils

    if "nc" not in _CACHE:
        _CACHE["nc"] = _build()
    nc = _CACHE["nc"]

    xt, wct, rh, eye = _prep_inputs(batch, W1, W2)
    in_maps = [
        {"xT": xt[i], "wct": wct, "rh": rh, "eye": eye} for i in range(NCORES)
    ]
    res = bass_utils.run_bass_kernel_spmd(
        nc, in_maps, core_ids=list(range(NCORES)), **_CACHE.get("run_kwargs", {})
    )
    _CACHE["last_result"] = res

    full = np.concatenate(
        [r["vout"].reshape(BP, H2, T) for r in res.results], axis=0
    )  # [104, 10, 2000]
    return np.ascontiguousarray(full.transpose(0, 2, 1)[:B].astype(np.float32))
